# revision 9
# baseline (speedup 1.0000x reference)
"""Trainium2 Bass kernel for nn_DiversityUncertainty (retrieval_knn).

out = lambda * norm01(entropy(pred)) + norm01(min_l2_dist(U_z, L_z))

Sharding: U_z / pred row-sharded across 8 cores (2048 rows each), L_z
replicated.  Per core:

  distance (orientation: l on partitions, u on free dim):
    - fp8 e4m3 DoubleRow GEMMs: psum[l, u] = (-2u_eff) . l_eff where
      u_eff = -(A8+R8)/2 (residual-quantized U), l_eff = B8 (+BR8).
      DoubleRow contracts K=256 in one matmul at 0.5 cyc/row, so the PE
      is never the bottleneck even at the 1.2GHz mid p-state.
    - evacuation ev = (256 - |l|^2) + 2u.l split between ScalarE
      (activation relu, per-partition bias) and DVE (tensor_scalar)
      chunks for engine balance; fp16 output.
    - running max over chunks (max ev <=> min d^2) split between DVE
      (2x fp16 tensor_tensor) and GpSimd chains (separate accumulators)
    - PE-transpose + free-dim reduce for the partition-axis max
    - d^2 stats collected pre-collective; sqrt + Newton hidden under the
      AllReduce latency.
  entropy: fp16 pred; ACT exp + DVE fp16 mul (2x) + reduce-add split
  across DVE/GpSimd.
  global min/max: one 4-scalar AllReduce(max) with negation packing.

Self-contained: all shapes hardcoded; no sibling imports.
"""

import numpy as np

# ---- problem constants (hardcoded per contract) ----
N_U, N_L, NZ, C = 16384, 8192, 256, 1000
CORES = 8
MU = N_U // CORES          # 2048 rows of U / pred per core
P = 128                    # partitions
MT = MU // P               # 16 m-tiles per core
NCH = N_L // P             # 64 n-chunks of 128
MMN = 512                  # moving free dim per matmul (1 psum bank fp32)
EPS = 1e-18
L2C = 256.0                # centering constant for the l2 bias
NMM = 3                    # fp8 matmuls per K=256 contraction (1..3)

# ---- engine-balance tunables ----
# evac engine per chunk: DVE for these chunk indices mod 10, ACT otherwise
DVE_EVAC_MOD = (2, 6)
# max-accumulate: GpSimd (Pool) for chunks < POOL_MAX_HI (its own acc
# chain), DVE for the rest.  Pool chunks come first so its queue drains
# before the collective trigger (which sits on the gpsimd queue).
POOL_MAX_HI = 30
# entropy reduce: ACT (Copy + accum_out) for tiles < ENT_RED_ACT, DVE rest
ENT_RED_ACT = 16
# entropy multiplies on GpSimd (Pool) to relieve DVE
ENT_MUL_POOL = True

_CACHE = {}


def _build(lam: float, legalize: bool = True):
    import concourse.bass as bass
    import concourse.tile as tile
    from concourse import mybir

    f32 = mybir.dt.float32
    f16 = mybir.dt.float16
    f8 = mybir.dt.float8e4
    AX = mybir.AxisListType
    OP = mybir.AluOpType
    AF = mybir.ActivationFunctionType
    DR = mybir.MatmulPerfMode.DoubleRow

    nc = bass.Bass(num_devices=CORES)

    # fp8 DoubleRow operands: [128 z-partitions, 2 k-subtiles, free]
    a8_h = nc.declare_dram_parameter("a8", [P, 2, MU], f8, isOutput=False)
    r8_h = (nc.declare_dram_parameter("r8", [P, 2, MU], f8, isOutput=False)
            if NMM >= 2 else None)
    b8_h = nc.declare_dram_parameter("b8", [P, 2, N_L], f8, isOutput=False)
    br8_h = (nc.declare_dram_parameter("br8", [P, 2, N_L], f8, isOutput=False)
             if NMM >= 3 else None)
    l2b_h = nc.declare_dram_parameter("l2b", [P, NCH], f32, isOutput=False)
    u2_h = nc.declare_dram_parameter("u2c", [P, MT], f32, isOutput=False)
    id_h = nc.declare_dram_parameter("idm", [P, P], f16, isOutput=False)
    id32_h = nc.declare_dram_parameter("idm32", [P, P], f32, isOutput=False)
    pr_h = nc.declare_dram_parameter("pred16", [MU, C], f16, isOutput=False)
    out_h = nc.declare_dram_parameter("outv", [P, MT], f32, isOutput=True)

    cc_in = nc.dram_tensor("cc_in", [4], f32)
    cc_out = nc.dram_tensor("cc_out", [4], f32, addr_space="Shared")

    from contextlib import ExitStack
    with tile.TileContext(nc) as tc, ExitStack() as stk:
        consts = stk.enter_context(tc.tile_pool(name="consts", bufs=1))
        preds = stk.enter_context(tc.tile_pool(name="preds", bufs=3))
        psums = stk.enter_context(tc.tile_pool(name="psums", bufs=2, space="PSUM"))
        evs = stk.enter_context(tc.tile_pool(name="evs", bufs=4))
        small = stk.enter_context(tc.tile_pool(name="small", bufs=1))

        # ---- resident SBUF tensors ----
        a8c = [consts.tile([P, 2, MMN], f8, tag=f"a8_{q}", name=f"a8_{q}") for q in range(4)]
        r8c = ([consts.tile([P, 2, MMN], f8, tag=f"r8_{q}", name=f"r8_{q}") for q in range(4)]
               if NMM >= 2 else None)
        LTW = N_L // 8
        b8c = [consts.tile([P, 2, LTW], f8, tag=f"b8_{q}", name=f"b8_{q}") for q in range(8)]
        br8c = ([consts.tile([P, 2, LTW], f8, tag=f"br8_{q}", name=f"br8_{q}") for q in range(8)]
                if NMM >= 3 else None)
        l2b = consts.tile([P, NCH], f32, tag="l2b")
        u2s = consts.tile([P, MT], f32, tag="u2s")
        idm = consts.tile([P, P], f16, tag="idm")
        idm32 = consts.tile([P, P], f32, tag="idm32")
        accD = consts.tile([P, MU], f16, tag="accD")   # DVE max chain
        accP = consts.tile([P, MU], f16, tag="accP")   # Pool max chain

        S = small.tile([P, MT], f32, tag="S")          # sum(exp(x)*x) per row
        maxT = small.tile([P, MT], f32, tag="maxT")

        # first-needed operands first: chunk 0's matmuls want b8[0], a8, r8
        nc.sync.dma_start(out=b8c[0], in_=b8_h[:, :, 0:LTW])
        for q in range(4):
            nc.sync.dma_start(out=a8c[q], in_=a8_h[:, :, q * MMN:(q + 1) * MMN])
        if NMM >= 2:
            for q in range(4):
                nc.sync.dma_start(out=r8c[q], in_=r8_h[:, :, q * MMN:(q + 1) * MMN])
        if NMM >= 3:
            nc.sync.dma_start(out=br8c[0], in_=br8_h[:, :, 0:LTW])
        nc.sync.dma_start(out=l2b, in_=l2b_h[:])
        nc.sync.dma_start(out=b8c[1], in_=b8_h[:, :, LTW:2 * LTW])
        if NMM >= 3:
            nc.sync.dma_start(out=br8c[1], in_=br8_h[:, :, LTW:2 * LTW])
        nc.sync.dma_start(out=u2s, in_=u2_h[:])
        nc.sync.dma_start(out=idm, in_=id_h[:])
        nc.sync.dma_start(out=idm32, in_=id32_h[:])

        # ---- distance: acc[p, m] = max over n-chunks of
        #      (C - l2[n]) + 2 u.l  with n = 128*chunk + p ----
        from concourse.tile import add_dep_helper
        chunk_mm = {}
        ent_t = 0
        for nb in range(NCH):
            ps = psums.tile([P, MU], f32, tag="ps")     # 4 banks
            first_mm = None
            bt = b8c[nb // 8][:, :, (nb % 8) * P:(nb % 8) * P + P]
            for s in range(MU // MMN):
                mm = nc.tensor.matmul(
                    ps[:, s * MMN:(s + 1) * MMN], bt, a8c[s],
                    start=True, stop=(NMM == 1), perf_mode=DR)
                if first_mm is None:
                    first_mm = mm
                    chunk_mm[nb] = mm
                if NMM >= 2:
                    nc.tensor.matmul(
                        ps[:, s * MMN:(s + 1) * MMN], bt, r8c[s],
                        start=False, stop=(NMM == 2), perf_mode=DR)
            if NMM >= 3:
                brt = br8c[nb // 8][:, :, (nb % 8) * P:(nb % 8) * P + P]
                for s in range(MU // MMN):
                    nc.tensor.matmul(
                        ps[:, s * MMN:(s + 1) * MMN], brt, a8c[s],
                        start=False, stop=True, perf_mode=DR)

            # stream in the b8 chunk needed 16 n-chunks from now, gated on
            # this chunk's first matmul so early DMA bandwidth goes to the
            # operands needed first
            if nb % 8 == 0 and nb // 8 + 2 < 8:
                q = nb // 8 + 2
                qs = slice(q * LTW, (q + 1) * LTW)
                d0 = nc.sync.dma_start(out=b8c[q], in_=b8_h[:, :, qs])
                add_dep_helper(d0.ins, first_mm.ins, sync=True,
                               reason="stage b8 behind compute")
                if NMM >= 3:
                    d1 = nc.sync.dma_start(out=br8c[q], in_=br8_h[:, :, qs])
                    add_dep_helper(d1.ins, first_mm.ins, sync=True,
                                   reason="stage br8 behind compute")

            use_dve_evac = (nb % 10) in DVE_EVAC_MOD
            # chunk 0 seeds accP (Pool chain), chunk POOL_MAX_HI seeds accD
            if nb == 0:
                nc.scalar.activation(accP, ps, AF.Relu,
                                     bias=l2b[:, 0:1], scale=-1.0)
            elif nb == POOL_MAX_HI:
                nc.scalar.activation(accD, ps, AF.Relu,
                                     bias=l2b[:, nb:nb + 1], scale=-1.0)
            else:
                ev = evs.tile([P, MU], f16, tag="ev")
                if use_dve_evac:
                    nc.vector.tensor_scalar(
                        out=ev, in0=ps, scalar1=l2b[:, nb:nb + 1],
                        scalar2=-1.0, op0=OP.subtract, op1=OP.mult)
                else:
                    nc.scalar.activation(ev, ps, AF.Relu,
                                         bias=l2b[:, nb:nb + 1], scale=-1.0)
                acc = accP if nb < POOL_MAX_HI else accD
                nc.vector.tensor_tensor(out=acc, in0=acc, in1=ev, op=OP.max)

            # ---- entropy interleave: tile t after chunk 4t+2 ----
            if nb % 4 == 2 and ent_t < MT:
                t = ent_t
                ent_t += 1
                pt = preds.tile([P, C], f16, tag="pt")
                pd = nc.sync.dma_start(out=pt, in_=pr_h[t * P:(t + 1) * P, :])
                if t >= 1:
                    gate = chunk_mm[min(4 * t, 60)]
                    add_dep_helper(pd.ins, gate.ins, sync=True,
                                   reason="stage pred behind distance")
                et = preds.tile([P, C], f16, tag="et")
                nc.scalar.activation(et, pt, AF.Exp)
                xe = preds.tile([P, C], f16, tag="xe")
                if ENT_MUL_POOL:
                    nc.gpsimd.tensor_tensor(out=xe, in0=et, in1=pt, op=OP.mult)
                else:
                    nc.vector.tensor_mul(xe, et, pt)
                if t < ENT_RED_ACT:
                    # ACT free-dim sum via activation Copy + accum_out;
                    # keeps the reduce off the (busier) DVE queue
                    edum = preds.tile([P, C], f16, tag="edum")
                    nc.scalar.activation(edum, xe, AF.Copy,
                                         accum_out=S[:, t:t + 1])
                else:
                    nc.vector.tensor_reduce(
                        out=S[:, t:t + 1], in_=xe, axis=AX.X, op=OP.add)
                if t == MT - 1:
                    # prefetch the sqrt table set (evicts exp's set); hides
                    # the ~2.7us ACT_TABLE_LOAD under distance chunks
                    sq_warm = small.tile([P, 1], f32, tag="sq_warm")
                    nc.scalar.activation(sq_warm, idm32[:, 0:1], AF.Sqrt)
                    # entropy stats (tail off the critical path)
                    sneg = small.tile([P, MT], f32, tag="sneg")
                    nc.vector.tensor_scalar_mul(sneg, S, -1.0)
                    STp = small.tile([P, 4], f32, tag="STp")
                    nc.vector.tensor_reduce(
                        out=STp[:, 0:1], in_=S, axis=AX.X, op=OP.max)
                    nc.vector.tensor_reduce(
                        out=STp[:, 1:2], in_=sneg, axis=AX.X, op=OP.max)

        # ---- partition-axis max: transpose both acc chains into one psum
        # tile, single reduce over [P, 2P] covers both ----
        for j in range(MT):
            tps = psums.tile([P, 2 * P], f16, tag="ps")
            nc.tensor.transpose(tps[:, 0:P], accD[:, j * P:(j + 1) * P], idm)
            nc.tensor.transpose(tps[:, P:2 * P], accP[:, j * P:(j + 1) * P], idm)
            nc.vector.tensor_reduce(
                out=maxT[:, j:j + 1], in_=tps, axis=AX.X, op=OP.max)

        # d^2 per row; stats in d^2 domain (sqrt is monotone)
        d2 = small.tile([P, MT], f32, tag="d2")
        nc.vector.tensor_sub(d2, u2s, maxT)             # (C+u2) - max = min d^2
        nc.vector.tensor_scalar_max(d2, d2, 1e-12)
        d2n = small.tile([P, MT], f32, tag="d2n")
        nc.vector.tensor_scalar_mul(d2n, d2, -1.0)
        nc.vector.tensor_reduce(out=STp[:, 2:3], in_=d2, axis=AX.X, op=OP.max)
        nc.vector.tensor_reduce(out=STp[:, 3:4], in_=d2n, axis=AX.X, op=OP.max)

        # partition-axis max of STp via PE transpose
        stps = psums.tile([4, P], f32, tag="ps")
        nc.tensor.transpose(stps, STp, idm32)
        STr = small.tile([4, 1], f32, tag="STr")
        nc.vector.tensor_reduce(out=STr, in_=stps, axis=AX.X, op=OP.max)

        nc.sync.dma_start(out=cc_in[:], in_=STr)
        nc.gpsimd.collective_compute(
            "AllReduce", OP.max,
            replica_groups=[list(range(CORES))],
            ins=[cc_in[:]], outs=[cc_out[:]],
        )

        # ---- hidden under the AllReduce: d = sqrt(d2) + one Newton step
        # (ACT sqrt table is low precision) ----
        dsq = small.tile([P, MT], f32, tag="dsq")
        nc.scalar.activation(dsq, d2, AF.Sqrt)
        rc = small.tile([P, MT], f32, tag="rc")
        nc.vector.reciprocal(rc, dsq)
        xy = small.tile([P, MT], f32, tag="xy")
        nc.vector.tensor_mul(xy, rc, d2)
        dv = small.tile([P, MT], f32, tag="dv")
        nc.vector.tensor_add(dv, dsq, xy)
        nc.vector.tensor_scalar_mul(dv, dv, 0.5)

        G = small.tile([P, 4], f32, tag="G")
        _co = cc_out[:]
        nc.sync.dma_start(out=G, in_=bass.AP(
            tensor=_co.tensor, offset=_co.offset,
            ap=[[0, P]] + [list(d) for d in _co.ap]))

        # ---- post-collective: G = [smax, -smin, d2max, -d2min] ----
        su = small.tile([P, 1], f32, tag="su")
        nc.vector.tensor_add(su, G[:, 0:1], G[:, 1:2])     # smax - smin
        nc.vector.tensor_scalar_add(su, su, EPS)
        ru = small.tile([P, 1], f32, tag="ru")
        nc.vector.reciprocal(ru, su)
        nc.vector.tensor_scalar_mul(ru, ru, -lam)          # -(lambda)/(span_u)

        # dmax/dmin = sqrt of collective d2 stats + one Newton step each,
        # done on [P, 2] at once: dd = [d2max, d2min]
        dd = small.tile([P, 2], f32, tag="dd")
        nc.vector.tensor_copy(out=dd[:, 0:1], in_=G[:, 2:3])
        nc.vector.tensor_scalar_mul(dd[:, 1:2], G[:, 3:4], -1.0)
        dds = small.tile([P, 2], f32, tag="dds")
        nc.scalar.activation(dds, dd, AF.Sqrt)
        ddr = small.tile([P, 2], f32, tag="ddr")
        nc.vector.reciprocal(ddr, dds)
        ddx = small.tile([P, 2], f32, tag="ddx")
        nc.vector.tensor_mul(ddx, ddr, dd)
        nc.vector.tensor_add(ddx, ddx, dds)
        nc.vector.tensor_scalar_mul(ddx, ddx, 0.5)         # [dmax, dmin]
        sd = small.tile([P, 1], f32, tag="sd")
        nc.vector.tensor_sub(sd, ddx[:, 0:1], ddx[:, 1:2])
        nc.vector.tensor_scalar_add(sd, sd, EPS)
        rd = small.tile([P, 1], f32, tag="rd")
        nc.vector.reciprocal(rd, sd)

        # out = lam*(smax - S)/span_u + (d - dmin)/span_d
        t1 = small.tile([P, MT], f32, tag="t1")
        nc.vector.tensor_scalar(
            out=t1, in0=S, scalar1=G[:, 0:1], scalar2=ru,
            op0=OP.subtract, op1=OP.mult)
        t2 = small.tile([P, MT], f32, tag="t2")
        nc.vector.tensor_scalar(
            out=t2, in0=dv, scalar1=ddx[:, 1:2], scalar2=rd,
            op0=OP.subtract, op1=OP.mult)
        ov = small.tile([P, MT], f32, tag="ov")
        nc.vector.tensor_add(ov, t1, t2)
        nc.sync.dma_start(out=out_h[:], in_=ov)

    _dedupe_ldweights(nc)
    if legalize:
        _split_multi_waits(nc, mybir)
    return nc


def _dedupe_ldweights(nc):
    """Consecutive PE matmuls over the same stationary tile each get their
    own InstLdweights from tile_legalize; the array state is unchanged, so
    drop the repeats (moving their sync info to the next PE instruction)."""
    import concourse.mybir as mybir
    PE = mybir.EngineType.PE
    for func in nc.m.functions:
        for block in func.blocks:
            out = []
            changed = False
            last_key = None
            pending = []            # sync entries from dropped LDWs
            for inst in block.instructions:
                if inst.engine != PE:
                    out.append(inst)
                    continue
                if isinstance(inst, mybir.InstLdweights):
                    key = str(inst.ins)
                    if key == last_key:
                        si = inst.sync_info
                        if si is not None:
                            pending.extend(list(si.on_wait or []))
                            pending.extend(
                                ("upd", u) for u in (si.on_update or []))
                        changed = True
                        continue
                    last_key = key
                if pending:
                    si = inst.sync_info
                    waits = list(si.on_wait or []) if si is not None else []
                    upds = list(si.on_update or []) if si is not None else []
                    for p in pending:
                        if isinstance(p, tuple):
                            upds.append(p[1])
                        else:
                            waits.append(p)
                    inst.sync_info = mybir.SyncInfo(on_wait=waits, on_update=upds)
                    pending = []
                out.append(inst)
            if changed:
                block.instructions = out


def _split_multi_waits(nc, mybir):
    """This walrus build accepts at most ONE sync-wait command per
    instruction; Tile freely attaches several.  Hoist all but the last
    wait onto dedicated same-engine NoOps inserted just before."""
    n = 0
    for func in nc.m.functions:
        for block in func.blocks:
            out = []
            changed = False
            for inst in block.instructions:
                si = inst.sync_info
                waits = list(si.on_wait) if si is not None and si.on_wait else []
                if len(waits) > 1:
                    for w in waits[:-1]:
                        nop = mybir.InstNoOp(name=f"WSPLIT-{n}", ins=[], outs=[])
                        n += 1
                        nop.engine = inst.engine
                        nop.sync_info = mybir.SyncInfo(on_wait=[w], on_update=[])
                        out.append(nop)
                    inst.sync_info = mybir.SyncInfo(
                        on_wait=[waits[-1]],
                        on_update=list(si.on_update or []))
                    changed = True
                out.append(inst)
            if changed:
                block.instructions = out


def _prep_inputs(pred, U_z, L_z):
    import ml_dtypes
    f = np.float32
    h = np.float16
    f8 = ml_dtypes.float8_e4m3
    pred = np.asarray(pred, dtype=f)
    U = np.asarray(U_z, dtype=f)
    L = np.asarray(L_z, dtype=f)

    # DoubleRow layout [128 z-part, 2 k-subtiles, free]
    def dr(x):                               # [rows, 256] -> [128, 2, rows]
        return np.ascontiguousarray(
            x.reshape(x.shape[0], 2, P).transpose(2, 1, 0))

    B = L.astype(f8)
    Bf = B.astype(f)
    l_eff = Bf.astype(np.float64)
    b8 = dr(B)
    br8 = None
    if NMM >= 3:
        BR = (L - Bf).astype(f8)
        br8 = dr(BR)
        l_eff = l_eff + BR.astype(np.float64)
    l2 = (l_eff * l_eff).sum(axis=1).astype(f)           # [N_L]
    l2bias = np.ascontiguousarray(
        (np.float32(L2C) - l2).reshape(NCH, P).T)        # [P, NCH]
    idm = np.eye(P, dtype=h)
    idm32 = np.eye(P, dtype=f)

    in_maps = []
    for c in range(CORES):
        r = slice(c * MU, (c + 1) * MU)
        Uc = U[r]
        A = (-2.0 * Uc).astype(f8)
        Af = A.astype(f)
        u_eff = Af.astype(np.float64)
        m = {"b8": b8, "l2b": l2bias, "idm": idm, "idm32": idm32,
             "pred16": np.ascontiguousarray(pred[r].astype(h)),
             "a8": dr(A)}
        if NMM >= 2:
            R = (-2.0 * Uc - Af).astype(f8)
            m["r8"] = dr(R)
            u_eff = u_eff + R.astype(np.float64)
        if NMM >= 3:
            m["br8"] = br8
        u_eff = u_eff / -2.0
        u2 = ((u_eff * u_eff).sum(axis=1)).astype(f) + np.float32(L2C)
        m["u2c"] = np.ascontiguousarray(u2.reshape(MT, P).T)
        in_maps.append(m)
    return in_maps


def _run(pred, U_z, L_z, lambda_, trace=False):
    from concourse import bass_utils
    lam = float(lambda_)
    key = lam
    if key not in _CACHE:
        _CACHE[key] = _build(lam)
    nc = _CACHE[key]
    in_maps = _prep_inputs(pred, U_z, L_z)
    res = bass_utils.run_bass_kernel_spmd(
        nc, in_maps, list(range(CORES)), trace=trace)
    out = np.empty(N_U, dtype=np.float32)
    for c in range(CORES):
        ov = res.results[c]["outv"]                      # [P, MT]
        out[c * MU:(c + 1) * MU] = ov.T.reshape(MU)
    return out, res


def kernel(pred, U_z, L_z, lambda_):
    out, _ = _run(pred, U_z, L_z, lambda_)
    return out


# revision 10
# speedup vs baseline: 1.1800x; 1.1800x over previous
"""Trainium2 Bass kernel for nn_DiversityUncertainty (retrieval_knn).

out = lambda * norm01(entropy(pred)) + norm01(min_l2_dist(U_z, L_z))

Sharding: U_z / pred row-sharded across 8 cores (2048 rows each), L_z
replicated.  Per core:

  distance (orientation: l on partitions, u on free dim):
    - fp8 e4m3 DoubleRow GEMMs: psum[l, u] = (-2u_eff) . l_eff where
      u_eff = -(A8+R8)/2 (residual-quantized U), l_eff = B8 (+BR8).
      DoubleRow contracts K=256 in one matmul at 0.5 cyc/row, so the PE
      is never the bottleneck even at the 1.2GHz mid p-state.
    - evacuation ev = (256 - |l|^2) + 2u.l split between ScalarE
      (activation relu, per-partition bias) and DVE (tensor_scalar)
      chunks for engine balance; fp16 output.
    - running max over chunks (max ev <=> min d^2) split between DVE
      (2x fp16 tensor_tensor) and GpSimd chains (separate accumulators)
    - PE-transpose + free-dim reduce for the partition-axis max
    - d^2 stats collected pre-collective; sqrt + Newton hidden under the
      AllReduce latency.
  entropy: fp16 pred; ACT exp + DVE fp16 mul (2x) + reduce-add split
  across DVE/GpSimd.
  global min/max: one 4-scalar AllReduce(max) with negation packing.

Self-contained: all shapes hardcoded; no sibling imports.
"""

import numpy as np

# ---- problem constants (hardcoded per contract) ----
N_U, N_L, NZ, C = 16384, 8192, 256, 1000
CORES = 8
MU = N_U // CORES          # 2048 rows of U / pred per core
P = 128                    # partitions
MT = MU // P               # 16 m-tiles per core
NCH = N_L // P             # 64 n-chunks of 128
MMN = 512                  # moving free dim per matmul (1 psum bank fp32)
EPS = 1e-18
L2C = 256.0                # centering constant for the l2 bias
NMM = 2                    # fp8 matmuls per K=256 contraction (1..3)

# ---- engine-balance tunables ----
# evac engine per chunk: DVE for these chunk indices mod 10, ACT otherwise
DVE_EVAC_MOD = (1, 5, 9)
# max-accumulate: GpSimd (Pool) for chunks < POOL_MAX_HI (its own acc
# chain), DVE for the rest.  Pool chunks come first so its queue drains
# before the collective trigger (which sits on the gpsimd queue).
POOL_MAX_HI = 30
# entropy reduce: ACT (Copy + accum_out) for tiles < ENT_RED_ACT, DVE rest
ENT_RED_ACT = 10
# entropy multiplies on GpSimd (Pool) to relieve DVE
ENT_MUL_POOL = True

_CACHE = {}


def _build(lam: float, legalize: bool = True):
    import concourse.bass as bass
    import concourse.tile as tile
    from concourse import mybir

    f32 = mybir.dt.float32
    f16 = mybir.dt.float16
    f8 = mybir.dt.float8e4
    AX = mybir.AxisListType
    OP = mybir.AluOpType
    AF = mybir.ActivationFunctionType
    DR = mybir.MatmulPerfMode.DoubleRow

    nc = bass.Bass(num_devices=CORES)

    # fp8 DoubleRow operands: [128 z-partitions, 2 k-subtiles, free]
    a8_h = nc.declare_dram_parameter("a8", [P, 2, MU], f8, isOutput=False)
    r8_h = (nc.declare_dram_parameter("r8", [P, 2, MU], f8, isOutput=False)
            if NMM >= 2 else None)
    b8_h = nc.declare_dram_parameter("b8", [P, 2, N_L], f8, isOutput=False)
    br8_h = (nc.declare_dram_parameter("br8", [P, 2, N_L], f8, isOutput=False)
             if NMM >= 3 else None)
    l2b_h = nc.declare_dram_parameter("l2b", [P, NCH], f32, isOutput=False)
    u2_h = nc.declare_dram_parameter("u2c", [P, MT], f32, isOutput=False)
    id_h = nc.declare_dram_parameter("idm", [P, P], f16, isOutput=False)
    id32_h = nc.declare_dram_parameter("idm32", [P, P], f32, isOutput=False)
    pr_h = nc.declare_dram_parameter("pred16", [MU, C], f16, isOutput=False)
    out_h = nc.declare_dram_parameter("outv", [P, MT], f32, isOutput=True)

    cc_in = nc.dram_tensor("cc_in", [4], f32)
    cc_out = nc.dram_tensor("cc_out", [4], f32, addr_space="Shared")

    from contextlib import ExitStack
    with tile.TileContext(nc) as tc, ExitStack() as stk:
        consts = stk.enter_context(tc.tile_pool(name="consts", bufs=1))
        preds = stk.enter_context(tc.tile_pool(name="preds", bufs=3))
        psums = stk.enter_context(tc.tile_pool(name="psums", bufs=2, space="PSUM"))
        evs = stk.enter_context(tc.tile_pool(name="evs", bufs=4))
        small = stk.enter_context(tc.tile_pool(name="small", bufs=1))

        # ---- resident SBUF tensors ----
        a8c = [consts.tile([P, 2, MMN], f8, tag=f"a8_{q}", name=f"a8_{q}") for q in range(4)]
        r8c = ([consts.tile([P, 2, MMN], f8, tag=f"r8_{q}", name=f"r8_{q}") for q in range(4)]
               if NMM >= 2 else None)
        LTW = N_L // 8
        b8c = [consts.tile([P, 2, LTW], f8, tag=f"b8_{q}", name=f"b8_{q}") for q in range(8)]
        br8c = ([consts.tile([P, 2, LTW], f8, tag=f"br8_{q}", name=f"br8_{q}") for q in range(8)]
                if NMM >= 3 else None)
        l2b = consts.tile([P, NCH], f32, tag="l2b")
        u2s = consts.tile([P, MT], f32, tag="u2s")
        idm = consts.tile([P, P], f16, tag="idm")
        idm32 = consts.tile([P, P], f32, tag="idm32")
        accD = consts.tile([P, MU], f16, tag="accD")   # DVE max chain
        accP = consts.tile([P, MU], f16, tag="accP")   # Pool max chain

        S = small.tile([P, MT], f32, tag="S")          # sum(exp(x)*x) per row
        maxT = small.tile([P, MT], f32, tag="maxT")

        # first-needed operands first: chunk 0's matmuls want b8[0], a8, r8
        nc.sync.dma_start(out=b8c[0], in_=b8_h[:, :, 0:LTW])
        for q in range(4):
            nc.sync.dma_start(out=a8c[q], in_=a8_h[:, :, q * MMN:(q + 1) * MMN])
        if NMM >= 2:
            for q in range(4):
                nc.sync.dma_start(out=r8c[q], in_=r8_h[:, :, q * MMN:(q + 1) * MMN])
        if NMM >= 3:
            nc.sync.dma_start(out=br8c[0], in_=br8_h[:, :, 0:LTW])
        nc.sync.dma_start(out=l2b, in_=l2b_h[:])
        nc.sync.dma_start(out=b8c[1], in_=b8_h[:, :, LTW:2 * LTW])
        if NMM >= 3:
            nc.sync.dma_start(out=br8c[1], in_=br8_h[:, :, LTW:2 * LTW])
        nc.sync.dma_start(out=u2s, in_=u2_h[:])
        nc.sync.dma_start(out=idm, in_=id_h[:])
        nc.sync.dma_start(out=idm32, in_=id32_h[:])

        # ---- distance: acc[p, m] = max over n-chunks of
        #      (C - l2[n]) + 2 u.l  with n = 128*chunk + p ----
        from concourse.tile import add_dep_helper
        chunk_mm = {}
        ent_t = 0
        for nb in range(NCH):
            ps = psums.tile([P, MU], f32, tag="ps")     # 4 banks
            first_mm = None
            bt = b8c[nb // 8][:, :, (nb % 8) * P:(nb % 8) * P + P]
            for s in range(MU // MMN):
                mm = nc.tensor.matmul(
                    ps[:, s * MMN:(s + 1) * MMN], bt, a8c[s],
                    start=True, stop=(NMM == 1), perf_mode=DR)
                if first_mm is None:
                    first_mm = mm
                    chunk_mm[nb] = mm
                if NMM >= 2:
                    nc.tensor.matmul(
                        ps[:, s * MMN:(s + 1) * MMN], bt, r8c[s],
                        start=False, stop=(NMM == 2), perf_mode=DR)
            if NMM >= 3:
                brt = br8c[nb // 8][:, :, (nb % 8) * P:(nb % 8) * P + P]
                for s in range(MU // MMN):
                    nc.tensor.matmul(
                        ps[:, s * MMN:(s + 1) * MMN], brt, a8c[s],
                        start=False, stop=True, perf_mode=DR)

            # stream in the b8 chunk needed 16 n-chunks from now, gated on
            # this chunk's first matmul so early DMA bandwidth goes to the
            # operands needed first
            if nb % 8 == 0 and nb // 8 + 2 < 8:
                q = nb // 8 + 2
                qs = slice(q * LTW, (q + 1) * LTW)
                d0 = nc.sync.dma_start(out=b8c[q], in_=b8_h[:, :, qs])
                add_dep_helper(d0.ins, first_mm.ins, sync=True,
                               reason="stage b8 behind compute")
                if NMM >= 3:
                    d1 = nc.sync.dma_start(out=br8c[q], in_=br8_h[:, :, qs])
                    add_dep_helper(d1.ins, first_mm.ins, sync=True,
                                   reason="stage br8 behind compute")

            use_dve_evac = (nb % 10) in DVE_EVAC_MOD
            # chunk 0 seeds accP (Pool chain), chunk POOL_MAX_HI seeds accD
            if nb == 0:
                nc.scalar.activation(accP, ps, AF.Relu,
                                     bias=l2b[:, 0:1], scale=-1.0)
            elif nb == POOL_MAX_HI:
                nc.scalar.activation(accD, ps, AF.Relu,
                                     bias=l2b[:, nb:nb + 1], scale=-1.0)
            else:
                ev = evs.tile([P, MU], f16, tag="ev")
                if use_dve_evac:
                    nc.vector.tensor_scalar(
                        out=ev, in0=ps, scalar1=l2b[:, nb:nb + 1],
                        scalar2=-1.0, op0=OP.subtract, op1=OP.mult)
                else:
                    nc.scalar.activation(ev, ps, AF.Relu,
                                         bias=l2b[:, nb:nb + 1], scale=-1.0)
                acc = accP if nb < POOL_MAX_HI else accD
                nc.vector.tensor_tensor(out=acc, in0=acc, in1=ev, op=OP.max)

            # ---- entropy interleave: tile t after chunk 4t+2 ----
            if nb % 4 == 2 and ent_t < MT:
                t = ent_t
                ent_t += 1
                pt = preds.tile([P, C], f16, tag="pt")
                pd = nc.sync.dma_start(out=pt, in_=pr_h[t * P:(t + 1) * P, :])
                if t >= 1:
                    gate = chunk_mm[min(4 * t, 60)]
                    add_dep_helper(pd.ins, gate.ins, sync=True,
                                   reason="stage pred behind distance")
                et = preds.tile([P, C], f16, tag="et")
                nc.scalar.activation(et, pt, AF.Exp)
                xe = preds.tile([P, C], f16, tag="xe")
                if ENT_MUL_POOL:
                    nc.gpsimd.tensor_tensor(out=xe, in0=et, in1=pt, op=OP.mult)
                else:
                    nc.vector.tensor_mul(xe, et, pt)
                if t < ENT_RED_ACT:
                    # ACT free-dim sum via activation Copy + accum_out;
                    # keeps the reduce off the (busier) DVE queue
                    edum = preds.tile([P, C], f16, tag="edum")
                    nc.scalar.activation(edum, xe, AF.Copy,
                                         accum_out=S[:, t:t + 1])
                else:
                    nc.vector.tensor_reduce(
                        out=S[:, t:t + 1], in_=xe, axis=AX.X, op=OP.add)
                if t == MT - 1:
                    # prefetch the sqrt table set (evicts exp's set); hides
                    # the ~2.7us ACT_TABLE_LOAD under distance chunks
                    sq_warm = small.tile([P, 1], f32, tag="sq_warm")
                    nc.scalar.activation(sq_warm, idm32[:, 0:1], AF.Sqrt)
                    # entropy stats (tail off the critical path)
                    sneg = small.tile([P, MT], f32, tag="sneg")
                    nc.vector.tensor_scalar_mul(sneg, S, -1.0)
                    STp = small.tile([P, 4], f32, tag="STp")
                    nc.vector.tensor_reduce(
                        out=STp[:, 0:1], in_=S, axis=AX.X, op=OP.max)
                    nc.vector.tensor_reduce(
                        out=STp[:, 1:2], in_=sneg, axis=AX.X, op=OP.max)

        # ---- partition-axis max: transpose both acc chains into one psum
        # tile, single reduce over [P, 2P] covers both ----
        for j in range(MT):
            tps = psums.tile([P, 2 * P], f16, tag="ps")
            nc.tensor.transpose(tps[:, 0:P], accD[:, j * P:(j + 1) * P], idm)
            nc.tensor.transpose(tps[:, P:2 * P], accP[:, j * P:(j + 1) * P], idm)
            nc.vector.tensor_reduce(
                out=maxT[:, j:j + 1], in_=tps, axis=AX.X, op=OP.max)

        # d^2 per row; stats in d^2 domain (sqrt is monotone)
        d2 = small.tile([P, MT], f32, tag="d2")
        nc.vector.tensor_sub(d2, u2s, maxT)             # (C+u2) - max = min d^2
        nc.vector.tensor_scalar_max(d2, d2, 1e-12)
        d2n = small.tile([P, MT], f32, tag="d2n")
        nc.vector.tensor_scalar_mul(d2n, d2, -1.0)
        nc.vector.tensor_reduce(out=STp[:, 2:3], in_=d2, axis=AX.X, op=OP.max)
        nc.vector.tensor_reduce(out=STp[:, 3:4], in_=d2n, axis=AX.X, op=OP.max)

        # partition-axis max of STp via PE transpose
        stps = psums.tile([4, P], f32, tag="ps")
        nc.tensor.transpose(stps, STp, idm32)
        STr = small.tile([4, 1], f32, tag="STr")
        nc.vector.tensor_reduce(out=STr, in_=stps, axis=AX.X, op=OP.max)

        nc.sync.dma_start(out=cc_in[:], in_=STr)
        nc.gpsimd.collective_compute(
            "AllReduce", OP.max,
            replica_groups=[list(range(CORES))],
            ins=[cc_in[:]], outs=[cc_out[:]],
        )

        # ---- hidden under the AllReduce: d = sqrt(d2) + one Newton step
        # (ACT sqrt table is low precision) ----
        dsq = small.tile([P, MT], f32, tag="dsq")
        nc.scalar.activation(dsq, d2, AF.Sqrt)
        rc = small.tile([P, MT], f32, tag="rc")
        nc.vector.reciprocal(rc, dsq)
        xy = small.tile([P, MT], f32, tag="xy")
        nc.vector.tensor_mul(xy, rc, d2)
        dv = small.tile([P, MT], f32, tag="dv")
        nc.vector.tensor_add(dv, dsq, xy)
        nc.vector.tensor_scalar_mul(dv, dv, 0.5)

        G = small.tile([P, 4], f32, tag="G")
        _co = cc_out[:]
        nc.sync.dma_start(out=G, in_=bass.AP(
            tensor=_co.tensor, offset=_co.offset,
            ap=[[0, P]] + [list(d) for d in _co.ap]))

        # ---- post-collective: G = [smax, -smin, d2max, -d2min] ----
        su = small.tile([P, 1], f32, tag="su")
        nc.vector.tensor_add(su, G[:, 0:1], G[:, 1:2])     # smax - smin
        nc.vector.tensor_scalar_add(su, su, EPS)
        ru = small.tile([P, 1], f32, tag="ru")
        nc.vector.reciprocal(ru, su)
        nc.vector.tensor_scalar_mul(ru, ru, -lam)          # -(lambda)/(span_u)

        # dmax/dmin = sqrt of collective d2 stats + one Newton step each,
        # done on [P, 2] at once: dd = [d2max, d2min]
        dd = small.tile([P, 2], f32, tag="dd")
        nc.vector.tensor_copy(out=dd[:, 0:1], in_=G[:, 2:3])
        nc.vector.tensor_scalar_mul(dd[:, 1:2], G[:, 3:4], -1.0)
        dds = small.tile([P, 2], f32, tag="dds")
        nc.scalar.activation(dds, dd, AF.Sqrt)
        ddr = small.tile([P, 2], f32, tag="ddr")
        nc.vector.reciprocal(ddr, dds)
        ddx = small.tile([P, 2], f32, tag="ddx")
        nc.vector.tensor_mul(ddx, ddr, dd)
        nc.vector.tensor_add(ddx, ddx, dds)
        nc.vector.tensor_scalar_mul(ddx, ddx, 0.5)         # [dmax, dmin]
        sd = small.tile([P, 1], f32, tag="sd")
        nc.vector.tensor_sub(sd, ddx[:, 0:1], ddx[:, 1:2])
        nc.vector.tensor_scalar_add(sd, sd, EPS)
        rd = small.tile([P, 1], f32, tag="rd")
        nc.vector.reciprocal(rd, sd)

        # out = lam*(smax - S)/span_u + (d - dmin)/span_d
        t1 = small.tile([P, MT], f32, tag="t1")
        nc.vector.tensor_scalar(
            out=t1, in0=S, scalar1=G[:, 0:1], scalar2=ru,
            op0=OP.subtract, op1=OP.mult)
        t2 = small.tile([P, MT], f32, tag="t2")
        nc.vector.tensor_scalar(
            out=t2, in0=dv, scalar1=ddx[:, 1:2], scalar2=rd,
            op0=OP.subtract, op1=OP.mult)
        ov = small.tile([P, MT], f32, tag="ov")
        nc.vector.tensor_add(ov, t1, t2)
        nc.sync.dma_start(out=out_h[:], in_=ov)

    _dedupe_ldweights(nc)
    if legalize:
        _split_multi_waits(nc, mybir)
    return nc


def _dedupe_ldweights(nc):
    """Consecutive PE matmuls over the same stationary tile each get their
    own InstLdweights from tile_legalize; the array state is unchanged, so
    drop the repeats (moving their sync info to the next PE instruction)."""
    import concourse.mybir as mybir
    PE = mybir.EngineType.PE
    for func in nc.m.functions:
        for block in func.blocks:
            out = []
            changed = False
            last_key = None
            pending = []            # sync entries from dropped LDWs
            for inst in block.instructions:
                if inst.engine != PE:
                    out.append(inst)
                    continue
                if isinstance(inst, mybir.InstLdweights):
                    key = str(inst.ins)
                    if key == last_key:
                        si = inst.sync_info
                        if si is not None:
                            pending.extend(list(si.on_wait or []))
                            pending.extend(
                                ("upd", u) for u in (si.on_update or []))
                        changed = True
                        continue
                    last_key = key
                if pending:
                    si = inst.sync_info
                    waits = list(si.on_wait or []) if si is not None else []
                    upds = list(si.on_update or []) if si is not None else []
                    for p in pending:
                        if isinstance(p, tuple):
                            upds.append(p[1])
                        else:
                            waits.append(p)
                    inst.sync_info = mybir.SyncInfo(on_wait=waits, on_update=upds)
                    pending = []
                out.append(inst)
            if changed:
                block.instructions = out


def _split_multi_waits(nc, mybir):
    """This walrus build accepts at most ONE sync-wait command per
    instruction; Tile freely attaches several.  Hoist all but the last
    wait onto dedicated same-engine NoOps inserted just before."""
    n = 0
    for func in nc.m.functions:
        for block in func.blocks:
            out = []
            changed = False
            for inst in block.instructions:
                si = inst.sync_info
                waits = list(si.on_wait) if si is not None and si.on_wait else []
                if len(waits) > 1:
                    for w in waits[:-1]:
                        nop = mybir.InstNoOp(name=f"WSPLIT-{n}", ins=[], outs=[])
                        n += 1
                        nop.engine = inst.engine
                        nop.sync_info = mybir.SyncInfo(on_wait=[w], on_update=[])
                        out.append(nop)
                    inst.sync_info = mybir.SyncInfo(
                        on_wait=[waits[-1]],
                        on_update=list(si.on_update or []))
                    changed = True
                out.append(inst)
            if changed:
                block.instructions = out


def _prep_inputs(pred, U_z, L_z):
    import ml_dtypes
    f = np.float32
    h = np.float16
    f8 = ml_dtypes.float8_e4m3
    pred = np.asarray(pred, dtype=f)
    U = np.asarray(U_z, dtype=f)
    L = np.asarray(L_z, dtype=f)

    # DoubleRow layout [128 z-part, 2 k-subtiles, free]
    def dr(x):                               # [rows, 256] -> [128, 2, rows]
        return np.ascontiguousarray(
            x.reshape(x.shape[0], 2, P).transpose(2, 1, 0))

    B = L.astype(f8)
    Bf = B.astype(f)
    l_eff = Bf.astype(np.float64)
    b8 = dr(B)
    br8 = None
    if NMM >= 3:
        BR = (L - Bf).astype(f8)
        br8 = dr(BR)
        l_eff = l_eff + BR.astype(np.float64)
    l2 = (l_eff * l_eff).sum(axis=1).astype(f)           # [N_L]
    l2bias = np.ascontiguousarray(
        (np.float32(L2C) - l2).reshape(NCH, P).T)        # [P, NCH]
    idm = np.eye(P, dtype=h)
    idm32 = np.eye(P, dtype=f)

    in_maps = []
    for c in range(CORES):
        r = slice(c * MU, (c + 1) * MU)
        Uc = U[r]
        A = (-2.0 * Uc).astype(f8)
        Af = A.astype(f)
        u_eff = Af.astype(np.float64)
        m = {"b8": b8, "l2b": l2bias, "idm": idm, "idm32": idm32,
             "pred16": np.ascontiguousarray(pred[r].astype(h)),
             "a8": dr(A)}
        if NMM >= 2:
            R = (-2.0 * Uc - Af).astype(f8)
            m["r8"] = dr(R)
            u_eff = u_eff + R.astype(np.float64)
        if NMM >= 3:
            m["br8"] = br8
        u_eff = u_eff / -2.0
        u2 = ((u_eff * u_eff).sum(axis=1)).astype(f) + np.float32(L2C)
        m["u2c"] = np.ascontiguousarray(u2.reshape(MT, P).T)
        in_maps.append(m)
    return in_maps


def _run(pred, U_z, L_z, lambda_, trace=False):
    from concourse import bass_utils
    lam = float(lambda_)
    key = lam
    if key not in _CACHE:
        _CACHE[key] = _build(lam)
    nc = _CACHE[key]
    in_maps = _prep_inputs(pred, U_z, L_z)
    res = bass_utils.run_bass_kernel_spmd(
        nc, in_maps, list(range(CORES)), trace=trace)
    out = np.empty(N_U, dtype=np.float32)
    for c in range(CORES):
        ov = res.results[c]["outv"]                      # [P, MT]
        out[c * MU:(c + 1) * MU] = ov.T.reshape(MU)
    return out, res


def kernel(pred, U_z, L_z, lambda_):
    out, _ = _run(pred, U_z, L_z, lambda_)
    return out
